# revision 25
# baseline (speedup 1.0000x reference)
"""Trainium2 Bass kernel for nn_Conv2d_22222160789797.

Conv2d: x [32,128,56,56] f32, weight [256,128,3,3] (OIHW), stride 1, pad 1
-> out [32,256,56,56] f32.

Strategy: data-parallel over batch across 8 cores (4 images/core), bf16
operands (rel err ~4e-3 vs the 2e-2 gate), and 1-D Winograd F(2,3) along W:
each pair of output columns costs 4 multiplies instead of 6, cutting PE rows
from 9 to 6 per output pixel (225.8K -> 150.5K rows/core, ~63us at
1 row/cyc @ 2.4GHz, vs ~94us for direct conv).

The Winograd input transform (d0-d2, d1+d2, d2-d1, d1-d3 over 2-strided
columns of the zero-padded 58-col rows) is precomputed on the HOST: it
doubles input DMA bytes (well within ring capacity) but keeps the on-device
vector engines under the PE's 2.0us/chunk cadence. Weights are
Winograd-transformed host-side too (Wt[ic,kh,pos,oc] = G @ w).

Per (img, half, 14-row chunk): 12 accumulating matmuls (3 kh taps x 4 wino
positions, contracting IC=128 on the partition dim) into 4 PSUM banks
m0..m3 (pos emission order [1,2,0,3]), then the output transform with
engine-legal PSUM access (DVE may read one PSUM operand per op, ACT one via
activation, GpSimd none), ordered so each PSUM bank is released as early as
possible:
  ACT: c1=m1, c3=m3 (PSUM->SBUF stages)
  DVE: v=c1-m2, u=c1+m2 (both right after m2 stops), out0(even)=u+m0
  GpSimd: out1(odd cols)=v-c3
Output DMAs alternate between the sync and gpsimd HWDGE queues (one queue
alone only drains ~186GB/s, which would put the 12.8MB/core of output on
the critical path); the ACT queue carries only c1/c3 so a DMA issue never
blocks the next chunk's staging copies. Input DMAs ride the sync queue,
except image-0's first band which goes on the gpsimd queue in parallel with
the weights. Host layouts are pos-major in emission order so every
transfer is contiguous per partition.

Head: image 0 arrives as 4 row-band DMAs (16 rows each) so chunk 0 gates on
one ~470KB transfer; 13 dummy matmuls bridge the DMA wait so the PE's HAM
clock gate is at full rate when real work starts.
"""

import numpy as np

import concourse.tile as tile
from concourse import bacc, mybir
from concourse.bass_utils import run_bass_kernel_spmd

N_CORES = 8
B, IC, H, W = 32, 128, 56, 56
OC, KH, KW = 256, 3, 3
BPC = B // N_CORES          # images per core
PH, PW = H + 2, W + 2       # padded 58x58
J = 28                      # winograd tiles per row (2 output cols each)
R = 14                      # output rows per chunk (PSUM: 14*28*4B = 1568B)
N_CHUNKS = H // R           # 4
OC_HALVES = OC // 128       # 2
BAND_ROWS = R + 2           # 16 padded rows cover one chunk's kh taps

_f32 = mybir.dt.float32
_bf16 = mybir.dt.bfloat16

_compiled_nc = None

N_WARMUP = 13


def _build(warmup=N_WARMUP):
    nc = bacc.Bacc("TRN2", target_bir_lowering=False, debug=False)
    # host-transformed input: [img, ic, padded row, pos*J]
    x_d = nc.dram_tensor("x", [BPC, IC, PH, 4 * J], _bf16,
                         kind="ExternalInput")
    w_d = nc.dram_tensor("w", [IC, KH * 4 * OC], _bf16, kind="ExternalInput")
    o_d = nc.dram_tensor("out", [BPC, OC, H, W], _f32, kind="ExternalOutput")
    # view for contiguous per-half weight loads: [ic, half, posblock, kh, 128]
    w5 = w_d[:].rearrange("p (h q k c) -> p h q k c", h=OC_HALVES, q=4,
                          k=KH, c=128)

    with tile.TileContext(nc) as tc:
        with (
            tc.tile_pool(name="w", bufs=1) as wpool,
            tc.tile_pool(name="x", bufs=1) as xpool,
            tc.tile_pool(name="o", bufs=4) as opool,
            tc.tile_pool(name="ps", bufs=8, space="PSUM") as pspool,
        ):
            if warmup:
                wscr = wpool.tile([128, 128], _bf16, name="wscr", tag="wscr")
                xscr = wpool.tile([128, R * J], _bf16, name="xscr",
                                  tag="xscr")
                nc.gpsimd.memset(wscr[:], 0.0)
                nc.gpsimd.memset(xscr[:], 0.0)
                for _ in range(warmup):
                    pwarm = pspool.tile([128, R, J], _f32, name="pwarm",
                                        tag="ps0", bufs=2)
                    nc.tensor.matmul(pwarm[:], wscr[:], xscr[:],
                                     start=True, stop=True)

            # weights per (half, AB-slice): each slice holds 2 pos blocks in
            # emission order, so chunk 0's first pos groups gate on a
            # quarter of the head bytes. POSMAP maps pos -> block index in
            # the host's [1,2,0,3]-ordered pos-major layout.
            whAB = []
            for half in range(OC_HALVES):
                a = wpool.tile([IC, 2, KH, 128], _bf16, name=f"wh{half}a",
                               tag=f"wh{half}a")
                bb = wpool.tile([IC, 2, KH, 128], _bf16, name=f"wh{half}b",
                                tag=f"wh{half}b")
                whAB.append((a, bb))

            # image 0 as 4 row-band tiles; band0 is split into its two pos
            # slices on the gpsimd ring, weights stream on the sync ring in
            # parallel.
            bands0 = []
            for ch in range(N_CHUNKS):
                b = xpool.tile([IC, BAND_ROWS, 4, J], _bf16, name="band",
                               tag="band", bufs=N_CHUNKS)
                bands0.append(b)
            nc.gpsimd.dma_start(bands0[0][:], x_d[0, :, 0:BAND_ROWS, :])
            nc.sync.dma_start(whAB[0][0][:], w5[:, 0, 0:2])
            nc.sync.dma_start(whAB[0][1][:], w5[:, 0, 2:4])
            nc.sync.dma_start(whAB[1][0][:], w5[:, 1, 0:2])
            nc.sync.dma_start(whAB[1][1][:], w5[:, 1, 2:4])
            for ch in range(1, N_CHUNKS):
                nc.sync.dma_start(
                    bands0[ch][:],
                    x_d[0, :, ch * R : ch * R + BAND_ROWS, :])

            # whole-image tiles for images 1..3, prefetched under compute
            # (bufs=3 so no DMA issue blocks on a ring reuse)
            xt = {}
            for img in range(1, BPC):
                xt[img] = xpool.tile([IC, PH, 4, J], _bf16, name="xt",
                                     tag="xt", bufs=3)
                nc.sync.dma_start(xt[img][:], x_d[img])

            POSMAP = {1: 0, 2: 1, 0: 2, 3: 3}

            def chunk_group(mv_fn, img, half, ch, out_ring):
                ps = {}

                def pos_group(pos):
                    p = pspool.tile([128, R, J], _f32, name=f"ps{pos}",
                                    tag=f"ps{pos}", bufs=2)
                    ps[pos] = p
                    q = POSMAP[pos]
                    wt = whAB[half][q // 2]
                    for kh in range(KH):
                        nc.tensor.matmul(
                            p[:],
                            wt[:, q % 2, kh, :],
                            mv_fn(pos, kh),
                            start=(kh == 0),
                            stop=(kh == KH - 1),
                        )

                ot = opool.tile([128, R, W], _f32, name="ot", tag="ot",
                                bufs=6)
                # pos order + eager consumer emission keeps every engine's
                # first op as close to its PSUM group's stop as possible.
                pos_group(1)
                c1 = opool.tile([128, R, J], _f32, name="c1", tag="c1",
                                bufs=3)
                nc.scalar.copy(c1[:], ps[1][:])
                pos_group(2)
                v = opool.tile([128, R, J], _f32, name="v", tag="v", bufs=3)
                nc.vector.tensor_sub(v[:], c1[:], ps[2][:])
                u = opool.tile([128, R, J], _f32, name="u", tag="u", bufs=3)
                nc.vector.tensor_add(u[:], c1[:], ps[2][:])
                pos_group(0)
                nc.vector.tensor_add(ot[:, :, 0 : 2 * J : 2], u[:],
                                     ps[0][:])
                pos_group(3)
                c3 = opool.tile([128, R, J], _f32, name="c3", tag="c3",
                                bufs=3)
                nc.scalar.copy(c3[:], ps[3][:])
                nc.gpsimd.tensor_sub(ot[:, :, 1 : 2 * J : 2], v[:], c3[:])
                out_ring.dma_start(
                    o_d[img, half * 128 : half * 128 + 128,
                        ch * R : ch * R + R, :],
                    ot[:],
                )

            def mv_band(ch):
                def f(pos, kh):
                    return bands0[ch][:, kh : kh + R, POSMAP[pos], :]
                return f

            def mv_img(img, ch):
                def f(pos, kh):
                    return xt[img][:, ch * R + kh : ch * R + kh + R,
                                   POSMAP[pos], :]
                return f

            slot = 0
            for img in range(BPC):
                for half in range(OC_HALVES):
                    for ch in range(N_CHUNKS):
                        ring = nc.gpsimd if slot % 2 == 0 else nc.sync
                        if img == 0:
                            mv = mv_band(ch)
                        else:
                            mv = mv_img(img, ch)
                        chunk_group(mv, img, half, ch, ring)
                        slot += 1
    nc.compile()
    return nc


def _get_nc():
    global _compiled_nc
    if _compiled_nc is None:
        _compiled_nc = _build()
    return _compiled_nc


def _prep_inputs(x, weight):
    import ml_dtypes

    x = np.asarray(x, dtype=np.float32)
    weight = np.asarray(weight, dtype=np.float32)
    xp = np.zeros((B, IC, PH, PW), dtype=np.float32)
    xp[:, :, 1 : H + 1, 1 : W + 1] = x
    # host winograd input transform along W: [B, IC, PH, 4, J]
    d0 = xp[:, :, :, 0 : 2 * J - 1 : 2]
    d1 = xp[:, :, :, 1 : 2 * J : 2]
    d2 = xp[:, :, :, 2 : 2 * J + 1 : 2]
    d3 = xp[:, :, :, 3 : 2 * J + 2 : 2]
    # pos blocks stored in emission order [1, 2, 0, 3]
    xt = np.empty((B, IC, PH, 4, J), dtype=ml_dtypes.bfloat16)
    xt[:, :, :, 0, :] = d1 + d2
    xt[:, :, :, 1, :] = d2 - d1
    xt[:, :, :, 2, :] = d0 - d2
    xt[:, :, :, 3, :] = d1 - d3
    xt = xt.reshape(B, IC, PH, 4 * J)
    # host winograd weight transform: Wt[ic, kh, pos, oc] = sum_kw G[pos,kw] w
    G = np.array(
        [[1, 0, 0], [0.5, 0.5, 0.5], [0.5, -0.5, 0.5], [0, 0, 1]],
        dtype=np.float32,
    )
    # [ic, kh, pos, oc] -> pos-major [ic, half, posblock, kh, 128] with
    # pos blocks in emission order [1, 2, 0, 3], so each (half, 2-pos
    # slice) is one contiguous DMA
    wt = (
        np.einsum("pw,oihw->ihpo", G, weight)
        .reshape(IC, KH, 4, OC_HALVES, 128)[:, :, [1, 2, 0, 3]]
        .transpose(0, 3, 2, 1, 4)
        .reshape(IC, KH * 4 * OC)
        .astype(ml_dtypes.bfloat16)
    )
    in_maps = [
        {"x": np.ascontiguousarray(xt[c * BPC : (c + 1) * BPC]), "w": wt}
        for c in range(N_CORES)
    ]
    return in_maps


def _run(x, weight, trace=False):
    nc = _get_nc()
    in_maps = _prep_inputs(x, weight)
    res = run_bass_kernel_spmd(nc, in_maps, list(range(N_CORES)), trace=trace)
    out = np.concatenate([res.results[c]["out"] for c in range(N_CORES)], axis=0)
    return out, res


def kernel(x, weight):
    out, _ = _run(x, weight)
    return out
